# revision 8
# baseline (speedup 1.0000x reference)
"""FPE router kernel for Trainium2 (8 NeuronCores, SPMD over tokens).

Math: the reference's FFT pipeline collapses algebraically. Circular
correlation with the fixed router memory R is multiplication by a circulant
matrix C (C[m, j] = R[(m - j) mod d]).  Row-l2norms only scale rows, so with

    U = proj_w.T @ C            [H, d]
    P = U @ l2norm(signatures).T  [H, K]

the scores are exactly

    scores = (x @ P) / sqrt(rowsum((x @ U)^2))

(the l2norm of x @ proj_w.T cancels between numerator and denominator, and
||S||^2 = ||l2norm(x W^T) @ C||^2 = rowsum((x @ U)^2) / ||x W^T||^2).
U and P depend only on the (replicated) router params and are precomputed on
host in float64 via FFTs.  The device does one [N,4096]x[4096,4096] matmul
(bf16 — the denominator averages over 4096 columns, so bf16 error ~2e-5
relative) for the denominator and one [N,4096]x[4096,64] fp32 matmul for the
numerator (which alone determines top-k ranking), then top-2 + softmax.
"""

import math

import numpy as np
import ml_dtypes

import concourse.bass as bass
from concourse import bacc
import concourse.mybir as mybir
from concourse.tile import TileContext
from concourse.bass_utils import run_bass_kernel_spmd

N_CORES = 8
N_TOKENS = 16384
HIDDEN = 4096
DIM = 4096
K_EXP = 64
NTOK = N_TOKENS // N_CORES  # tokens per core
F32 = mybir.dt.float32
BF16 = mybir.dt.bfloat16
U32 = mybir.dt.uint32

_PROGRAM_CACHE = {}
LAST_RESULTS = None  # BassKernelResults of the most recent run (for test.py)


def _host_precompute(proj_w, positions, theta, signatures):
    """U = proj_w.T @ C and P = U @ E.T, in float64 via FFTs."""
    W = proj_w.astype(np.float64)
    pos = positions.astype(np.float64)
    th = theta.astype(np.float64)
    sig = signatures.astype(np.float64)
    d = th.shape[-1]

    E = sig / np.maximum(np.linalg.norm(sig, axis=-1, keepdims=True), 1e-12)
    L = np.fft.ifft(np.exp(1j * pos[:, None] * th[None, :]), axis=-1).real
    R = np.fft.irfft(
        np.fft.rfft(E, axis=-1) * np.fft.rfft(L, axis=-1), n=d, axis=-1
    ).sum(axis=0)
    # row w of (W.T @ C): (w @ C)[j] = sum_m w[m] R[(m-j) mod d]
    #                               = irfft(rfft(w) * conj(rfft(R)))
    U = np.fft.irfft(
        np.fft.rfft(W.T, axis=-1) * np.conj(np.fft.rfft(R))[None, :], n=d, axis=-1
    )
    P = U @ E.T
    return U, P


def build_program(ntok=NTOK):
    """One NeuronCore's program; identical across the 8 cores (SPMD)."""
    D = DIM
    KT = D // 128          # contraction chunks
    TT = ntok // 128       # token tiles
    CT = D // 512          # 512-wide column tiles of U
    BLK = min(4, TT)       # token tiles per denominator block
    NBLK = TT // BLK
    NNUM = math.ceil(TT * K_EXP / 512)  # PSUM banks holding the numerator

    nc = bacc.Bacc(None, target_bir_lowering=False)
    xt = nc.dram_tensor("xt", [D, ntok], F32, kind="ExternalInput")
    u = nc.dram_tensor("u", [D, D], BF16, kind="ExternalInput")
    # P rearranged on host to [128, KT*64]: chunk k lives at [:, k*64:(k+1)*64]
    p = nc.dram_tensor("p", [128, KT * K_EXP], F32, kind="ExternalInput")
    scores_o = nc.dram_tensor("scores", [ntok, K_EXP], F32, kind="ExternalOutput")
    weights_o = nc.dram_tensor("weights", [ntok, 2], F32, kind="ExternalOutput")
    indices_o = nc.dram_tensor("indices", [ntok, 2], U32, kind="ExternalOutput")

    with TileContext(nc) as tc:
        with (
            tc.tile_pool(name="persist", bufs=1) as persist,
            tc.tile_pool(name="slab", bufs=3) as slab_pool,
            tc.tile_pool(name="ustream", bufs=6) as u_pool,
            tc.tile_pool(name="small", bufs=4) as small,
        ):
            # resident: bf16 copy of x.T shard, P, numerator, square scratch
            xb = persist.tile([128, KT, ntok], BF16)
            p_sb = persist.tile([128, KT * K_EXP], F32)
            num_sb = persist.tile([128, TT * K_EXP], F32)
            sq = persist.tile([128, 512], F32)

            nc.sync.dma_start(p_sb[:], p[:])

            # ---- phase 0: stream x.T slabs; fp32 numerator MMs; cast to bf16
            with tc.tile_pool(name="psum_num", bufs=1, space="PSUM") as psum_num:
                num_ps = [
                    psum_num.tile([128, 512], F32, tag=f"np{i}", name=f"np{i}")
                    for i in range(NNUM)
                ]
                for k in range(KT):
                    s32 = slab_pool.tile([128, ntok], F32, tag="s32")
                    nc.sync.dma_start(s32[:], xt[k * 128:(k + 1) * 128, :])
                    nc.vector.tensor_copy(xb[:, k, :], s32[:])
                    for t in range(TT):
                        # 8 token tiles share one PSUM bank: start/stop only on
                        # the bank's first/last matmul (start marks the whole
                        # 2KB zero region; later writes overwrite-then-
                        # accumulate per element)
                        nc.tensor.matmul(
                            num_ps[t // 8][:, (t % 8) * K_EXP:(t % 8 + 1) * K_EXP],
                            s32[:, t * 128:(t + 1) * 128],
                            p_sb[:, k * K_EXP:(k + 1) * K_EXP],
                            start=(k == 0 and t % 8 == 0),
                            stop=(k == KT - 1 and (t % 8 == 7 or t == TT - 1)),
                        )
                for i in range(NNUM):
                    w = min(512, TT * K_EXP - i * 512)
                    nc.vector.tensor_copy(
                        num_sb[:, i * 512:i * 512 + w], num_ps[i][:, :w]
                    )

            # ---- phase 1: bf16 denominator matmul + epilogue per token tile
            psum_d_cm = tc.tile_pool(name="psum_d", bufs=8 // BLK, space="PSUM")
            psum_d = psum_d_cm.__enter__()
            for blk in range(NBLK):
                dparts = [
                    small.tile([128, CT], F32, tag=f"dp{t}", name=f"dp{t}") for t in range(BLK)
                ]
                for col in range(CT):
                    pss = [
                        psum_d.tile([128, 512], F32, tag=f"ps{t}", name=f"ps{t}") for t in range(BLK)
                    ]
                    for k in range(KT):
                        ut = u_pool.tile([128, 512], BF16, tag="ut")
                        nc.sync.dma_start(
                            ut[:], u[k * 128:(k + 1) * 128, col * 512:(col + 1) * 512]
                        )
                        for t in range(BLK):
                            tok = blk * BLK + t
                            nc.tensor.matmul(
                                pss[t][:],
                                xb[:, k, tok * 128:(tok + 1) * 128],
                                ut[:],
                                start=(k == 0),
                                stop=(k == KT - 1),
                            )
                    for t in range(BLK):
                        # sum of squares along free dim -> dparts[t][:, col]
                        nc.scalar.activation(
                            sq[:],
                            pss[t][:],
                            mybir.ActivationFunctionType.Square,
                            accum_out=dparts[t][:, col:col + 1],
                        )
                for t in range(BLK):
                    tok = blk * BLK + t
                    den = small.tile([128, 1], F32, tag="den")
                    nc.vector.tensor_reduce(
                        den[:], dparts[t][:], axis=mybir.AxisListType.X,
                        op=mybir.AluOpType.add,
                    )
                    nc.scalar.sqrt(den[:], den[:])
                    rden = small.tile([128, 1], F32, tag="rden")
                    nc.vector.reciprocal(rden[:], den[:])
                    sc_t = small.tile([128, K_EXP], F32, tag="sc")
                    nc.vector.tensor_scalar_mul(
                        sc_t[:], num_sb[:, tok * K_EXP:(tok + 1) * K_EXP], rden[:]
                    )
                    nc.sync.dma_start(
                        scores_o[tok * 128:(tok + 1) * 128, :], sc_t[:]
                    )
                    m8 = small.tile([128, 8], F32, tag="m8")
                    i8 = small.tile([128, 8], U32, tag="i8")
                    nc.vector.max(m8[:], sc_t[:])
                    nc.vector.max_index(i8[:], m8[:], sc_t[:])
                    nc.sync.dma_start(
                        indices_o[tok * 128:(tok + 1) * 128, :], i8[:, 0:2]
                    )
                    dif = small.tile([128, 1], F32, tag="dif")
                    nc.vector.tensor_sub(dif[:], m8[:, 1:2], m8[:, 0:1])  # v1 - v0
                    wt = small.tile([128, 2], F32, tag="wt")
                    nc.scalar.activation(
                        wt[:, 0:1], dif[:], mybir.ActivationFunctionType.Sigmoid,
                        scale=-1.0,
                    )
                    nc.scalar.activation(
                        wt[:, 1:2], dif[:], mybir.ActivationFunctionType.Sigmoid,
                    )
                    nc.sync.dma_start(
                        weights_o[tok * 128:(tok + 1) * 128, :], wt[:]
                    )
            psum_d_cm.__exit__(None, None, None)
    nc.compile()
    return nc


def kernel(x, proj_w, positions, theta, signatures):
    global LAST_RESULTS
    U, P = _host_precompute(proj_w, positions, theta, signatures)

    u_bf16 = np.ascontiguousarray(U.astype(ml_dtypes.bfloat16))
    KT = DIM // 128
    p_re = np.ascontiguousarray(
        P.astype(np.float32).reshape(KT, 128, K_EXP).transpose(1, 0, 2)
        .reshape(128, KT * K_EXP)
    )
    xt_full = np.ascontiguousarray(x.astype(np.float32).T)  # [HIDDEN, N_TOKENS]

    key = NTOK
    if key not in _PROGRAM_CACHE:
        _PROGRAM_CACHE[key] = build_program(NTOK)
    nc = _PROGRAM_CACHE[key]

    in_maps = []
    for c in range(N_CORES):
        sl = np.ascontiguousarray(xt_full[:, c * NTOK:(c + 1) * NTOK])
        in_maps.append({"xt": sl, "u": u_bf16, "p": p_re})

    LAST_RESULTS = run_bass_kernel_spmd(nc, in_maps, list(range(N_CORES)))
    results = LAST_RESULTS.results

    weights = np.concatenate([r["weights"] for r in results], axis=0)
    indices = np.concatenate(
        [r["indices"].astype(np.int32) for r in results], axis=0
    )
    scores = np.concatenate([r["scores"] for r in results], axis=0)
    return weights.astype(np.float32), indices, scores.astype(np.float32)


# revision 9
# speedup vs baseline: 69.6203x; 69.6203x over previous
"""FPE router kernel for Trainium2 (8 NeuronCores, SPMD over tokens).

Math: the reference's FFT pipeline collapses algebraically. Circular
correlation with the fixed router memory R is multiplication by a circulant
matrix C (C[m, j] = R[(m - j) mod d]).  Row-l2norms only scale rows, so with

    U = proj_w.T @ C            [H, d]
    P = U @ l2norm(signatures).T  [H, K]

the scores are exactly

    scores = (x @ P) / sqrt(rowsum((x @ U)^2))

(the l2norm of x @ proj_w.T cancels between numerator and denominator, and
||S||^2 = ||l2norm(x W^T) @ C||^2 = rowsum((x @ U)^2) / ||x W^T||^2).
U and P depend only on the (replicated) router params and are precomputed on
host in float64 via FFTs.  The device does one [N,4096]x[4096,4096] matmul
(bf16 — the denominator averages over 4096 columns, so bf16 error ~2e-5
relative) for the denominator and one [N,4096]x[4096,64] fp32 matmul for the
numerator (which alone determines top-k ranking), then top-2 + softmax.
"""

import math

import numpy as np
import ml_dtypes

import concourse.bass as bass
from concourse import bacc
import concourse.mybir as mybir
from concourse.tile import TileContext
from concourse.bass_utils import run_bass_kernel_spmd

N_CORES = 8
N_TOKENS = 16384
HIDDEN = 4096
DIM = 4096
K_EXP = 64
NTOK = N_TOKENS // N_CORES  # tokens per core
F32 = mybir.dt.float32
BF16 = mybir.dt.bfloat16
U32 = mybir.dt.uint32

_PROGRAM_CACHE = {}
LAST_RESULTS = None  # BassKernelResults of the most recent run (for test.py)


def _host_precompute(proj_w, positions, theta, signatures):
    """U = proj_w.T @ C and P = U @ E.T, in float64 via FFTs."""
    W = proj_w.astype(np.float64)
    pos = positions.astype(np.float64)
    th = theta.astype(np.float64)
    sig = signatures.astype(np.float64)
    d = th.shape[-1]

    E = sig / np.maximum(np.linalg.norm(sig, axis=-1, keepdims=True), 1e-12)
    L = np.fft.ifft(np.exp(1j * pos[:, None] * th[None, :]), axis=-1).real
    R = np.fft.irfft(
        np.fft.rfft(E, axis=-1) * np.fft.rfft(L, axis=-1), n=d, axis=-1
    ).sum(axis=0)
    # row w of (W.T @ C): (w @ C)[j] = sum_m w[m] R[(m-j) mod d]
    #                               = irfft(rfft(w) * conj(rfft(R)))
    U = np.fft.irfft(
        np.fft.rfft(W.T, axis=-1) * np.conj(np.fft.rfft(R))[None, :], n=d, axis=-1
    )
    P = U @ E.T
    return U, P


def build_program(ntok=NTOK, nrep=1):
    """One NeuronCore's program; identical across the 8 cores (SPMD).

    nrep > 1 wraps the whole computation in an on-device loop (used only by
    test.py to measure per-iteration HW time through the axon tunnel, whose
    per-execute overhead is ~100x the kernel itself)."""
    D = DIM
    KT = D // 128          # contraction chunks
    TT = ntok // 128       # token tiles
    CT = D // 512          # 512-wide column tiles of U
    BLK = min(4, TT)       # token tiles per denominator block
    NBLK = TT // BLK
    NNUM = math.ceil(TT * K_EXP / 512)  # PSUM banks holding the numerator

    nc = bacc.Bacc(None, target_bir_lowering=False)
    xt = nc.dram_tensor("xt", [D, ntok], F32, kind="ExternalInput")
    u = nc.dram_tensor("u", [D, D], BF16, kind="ExternalInput")
    # P rearranged on host to [128, KT*64]: chunk k lives at [:, k*64:(k+1)*64]
    p = nc.dram_tensor("p", [128, KT * K_EXP], F32, kind="ExternalInput")
    scores_o = nc.dram_tensor("scores", [ntok, K_EXP], F32, kind="ExternalOutput")
    weights_o = nc.dram_tensor("weights", [ntok, 2], F32, kind="ExternalOutput")
    indices_o = nc.dram_tensor("indices", [ntok, 2], U32, kind="ExternalOutput")

    with TileContext(nc) as tc:
        with (
            tc.tile_pool(name="persist", bufs=1) as persist,
            tc.tile_pool(name="slab", bufs=3) as slab_pool,
            tc.tile_pool(name="ustream", bufs=6) as u_pool,
            tc.tile_pool(name="small", bufs=4) as small,
        ):
            # resident: bf16 copy of x.T shard, P, numerator, square scratch
            xb = persist.tile([128, KT, ntok], BF16)
            p_sb = persist.tile([128, KT * K_EXP], F32)
            num_sb = persist.tile([128, TT * K_EXP], F32)
            sq = persist.tile([128, 512], F32)

            loop_cm = tc.For_i(0, nrep, 1) if nrep > 1 else None
            if loop_cm is not None:
                loop_cm.__enter__()

            nc.sync.dma_start(p_sb[:], p[:])

            # ---- phase 0: stream x.T slabs; fp32 numerator MMs; cast to bf16
            with tc.tile_pool(name="psum_num", bufs=1, space="PSUM") as psum_num:
                num_ps = [
                    psum_num.tile([128, 512], F32, tag=f"np{i}", name=f"np{i}")
                    for i in range(NNUM)
                ]
                for k in range(KT):
                    s32 = slab_pool.tile([128, ntok], F32, tag="s32")
                    nc.sync.dma_start(s32[:], xt[k * 128:(k + 1) * 128, :])
                    nc.vector.tensor_copy(xb[:, k, :], s32[:])
                    for t in range(TT):
                        # 8 token tiles share one PSUM bank: start/stop only on
                        # the bank's first/last matmul (start marks the whole
                        # 2KB zero region; later writes overwrite-then-
                        # accumulate per element)
                        nc.tensor.matmul(
                            num_ps[t // 8][:, (t % 8) * K_EXP:(t % 8 + 1) * K_EXP],
                            s32[:, t * 128:(t + 1) * 128],
                            p_sb[:, k * K_EXP:(k + 1) * K_EXP],
                            start=(k == 0 and t % 8 == 0),
                            stop=(k == KT - 1 and (t % 8 == 7 or t == TT - 1)),
                        )
                for i in range(NNUM):
                    w = min(512, TT * K_EXP - i * 512)
                    nc.vector.tensor_copy(
                        num_sb[:, i * 512:i * 512 + w], num_ps[i][:, :w]
                    )

            # ---- phase 1: bf16 denominator matmul + epilogue per token tile
            psum_d_cm = tc.tile_pool(name="psum_d", bufs=8 // BLK, space="PSUM")
            psum_d = psum_d_cm.__enter__()
            for blk in range(NBLK):
                dparts = [
                    small.tile([128, CT], F32, tag=f"dp{t}", name=f"dp{t}") for t in range(BLK)
                ]
                for col in range(CT):
                    pss = [
                        psum_d.tile([128, 512], F32, tag=f"ps{t}", name=f"ps{t}") for t in range(BLK)
                    ]
                    for k in range(KT):
                        ut = u_pool.tile([128, 512], BF16, tag="ut")
                        nc.sync.dma_start(
                            ut[:], u[k * 128:(k + 1) * 128, col * 512:(col + 1) * 512]
                        )
                        for t in range(BLK):
                            tok = blk * BLK + t
                            nc.tensor.matmul(
                                pss[t][:],
                                xb[:, k, tok * 128:(tok + 1) * 128],
                                ut[:],
                                start=(k == 0),
                                stop=(k == KT - 1),
                            )
                    for t in range(BLK):
                        # sum of squares along free dim -> dparts[t][:, col]
                        nc.scalar.activation(
                            sq[:],
                            pss[t][:],
                            mybir.ActivationFunctionType.Square,
                            accum_out=dparts[t][:, col:col + 1],
                        )
                for t in range(BLK):
                    tok = blk * BLK + t
                    den = small.tile([128, 1], F32, tag="den")
                    nc.vector.tensor_reduce(
                        den[:], dparts[t][:], axis=mybir.AxisListType.X,
                        op=mybir.AluOpType.add,
                    )
                    nc.scalar.sqrt(den[:], den[:])
                    rden = small.tile([128, 1], F32, tag="rden")
                    nc.vector.reciprocal(rden[:], den[:])
                    sc_t = small.tile([128, K_EXP], F32, tag="sc")
                    nc.vector.tensor_scalar_mul(
                        sc_t[:], num_sb[:, tok * K_EXP:(tok + 1) * K_EXP], rden[:]
                    )
                    nc.sync.dma_start(
                        scores_o[tok * 128:(tok + 1) * 128, :], sc_t[:]
                    )
                    m8 = small.tile([128, 8], F32, tag="m8")
                    i8 = small.tile([128, 8], U32, tag="i8")
                    nc.vector.max(m8[:], sc_t[:])
                    nc.vector.max_index(i8[:], m8[:], sc_t[:])
                    nc.sync.dma_start(
                        indices_o[tok * 128:(tok + 1) * 128, :], i8[:, 0:2]
                    )
                    dif = small.tile([128, 1], F32, tag="dif")
                    nc.vector.tensor_sub(dif[:], m8[:, 1:2], m8[:, 0:1])  # v1 - v0
                    wt = small.tile([128, 2], F32, tag="wt")
                    nc.scalar.activation(
                        wt[:, 0:1], dif[:], mybir.ActivationFunctionType.Sigmoid,
                        scale=-1.0,
                    )
                    nc.scalar.activation(
                        wt[:, 1:2], dif[:], mybir.ActivationFunctionType.Sigmoid,
                    )
                    nc.sync.dma_start(
                        weights_o[tok * 128:(tok + 1) * 128, :], wt[:]
                    )
            psum_d_cm.__exit__(None, None, None)
            if loop_cm is not None:
                loop_cm.__exit__(None, None, None)
    nc.compile()
    return nc


def kernel(x, proj_w, positions, theta, signatures):
    global LAST_RESULTS
    U, P = _host_precompute(proj_w, positions, theta, signatures)

    u_bf16 = np.ascontiguousarray(U.astype(ml_dtypes.bfloat16))
    KT = DIM // 128
    p_re = np.ascontiguousarray(
        P.astype(np.float32).reshape(KT, 128, K_EXP).transpose(1, 0, 2)
        .reshape(128, KT * K_EXP)
    )
    xt_full = np.ascontiguousarray(x.astype(np.float32).T)  # [HIDDEN, N_TOKENS]

    key = (NTOK, 1)
    if key not in _PROGRAM_CACHE:
        _PROGRAM_CACHE[key] = build_program(NTOK)
    nc = _PROGRAM_CACHE[key]

    in_maps = []
    for c in range(N_CORES):
        sl = np.ascontiguousarray(xt_full[:, c * NTOK:(c + 1) * NTOK])
        in_maps.append({"xt": sl, "u": u_bf16, "p": p_re})

    LAST_RESULTS = run_bass_kernel_spmd(nc, in_maps, list(range(N_CORES)))
    results = LAST_RESULTS.results

    weights = np.concatenate([r["weights"] for r in results], axis=0)
    indices = np.concatenate(
        [r["indices"].astype(np.int32) for r in results], axis=0
    )
    scores = np.concatenate([r["scores"] for r in results], axis=0)
    return weights.astype(np.float32), indices, scores.astype(np.float32)


# revision 15
# speedup vs baseline: 147.0585x; 2.1123x over previous
"""FPE router kernel for Trainium2 (8 NeuronCores, SPMD over tokens).

Math: the reference's FFT pipeline collapses algebraically. Circular
correlation with the fixed router memory R is multiplication by a circulant
matrix C (C[m, j] = R[(m - j) mod d]).  Row-l2norms only scale rows, so with

    U = proj_w.T @ C              [H, d]
    P = U @ l2norm(signatures).T  [H, K]

the scores are exactly

    scores = (x @ P) / sqrt(rowsum((x @ U)^2))

(the l2norm of x @ proj_w.T cancels between numerator and denominator, and
||S|| = ||l2norm(x W^T) @ C|| = sqrt(rowsum((x @ U)^2)) / ||x W^T||).
U and P depend only on the (replicated) router params and are precomputed on
host in float64 via FFTs.

Device (per core, 2048 tokens):
  - numerator x @ P in fp32 on the tensor engine (it alone determines the
    top-k ranking; min gap between 2nd/3rd scores is ~2e-7 so it needs full
    precision), computed transposed (P as stationary, x.T slabs as moving)
    so it hides under the x DMA, then PE-transposed back;
  - denominator matmul x @ U in fp8e4 with DoubleRow (the row-sum of squares
    averages 4096 columns, so fp8 error ~1e-3 relative survives), scales
    16 (x) and 256 (U) keep values well inside fp8e4's +-240 range;
  - sum-of-squares on the scalar engine, sqrt (descaled) + reciprocal,
    top-2 via the vector engine's max/max_index, softmax as two sigmoids.
"""

import math

import numpy as np
import ml_dtypes

import concourse.bass as bass
from concourse import bacc
import concourse.mybir as mybir
from concourse.tile import TileContext
from concourse.bass_utils import run_bass_kernel_spmd
from concourse.masks import make_identity

N_CORES = 8
N_TOKENS = 16384
HIDDEN = 4096
DIM = 4096
K_EXP = 64
NTOK = N_TOKENS // N_CORES  # tokens per core
F32 = mybir.dt.float32
FP8 = mybir.dt.float8e4
U32 = mybir.dt.uint32

S_X = 16.0   # fp8 scale for x
S_U = 256.0  # fp8 scale for U

_PROGRAM_CACHE = {}
LAST_RESULTS = None  # BassKernelResults of the most recent run (for test.py)


def _host_precompute(proj_w, positions, theta, signatures):
    """U = proj_w.T @ C and P = U @ E.T, in float64 via FFTs."""
    W = proj_w.astype(np.float64)
    pos = positions.astype(np.float64)
    th = theta.astype(np.float64)
    sig = signatures.astype(np.float64)
    d = th.shape[-1]

    E = sig / np.maximum(np.linalg.norm(sig, axis=-1, keepdims=True), 1e-12)
    L = np.fft.ifft(np.exp(1j * pos[:, None] * th[None, :]), axis=-1).real
    R = np.fft.irfft(
        np.fft.rfft(E, axis=-1) * np.fft.rfft(L, axis=-1), n=d, axis=-1
    ).sum(axis=0)
    # row w of (W.T @ C): (w @ C)[j] = sum_m w[m] R[(m-j) mod d]
    #                               = irfft(rfft(w) * conj(rfft(R)))
    U = np.fft.irfft(
        np.fft.rfft(W.T, axis=-1) * np.conj(np.fft.rfft(R))[None, :], n=d, axis=-1
    )
    P = U @ E.T
    return U, P


def host_arrays(U, P):
    """Device-layout arrays shared by all cores: fp8 DoubleRow U, fp32 P."""
    KT = DIM // 128
    fp8_np = mybir.dt.np(FP8)
    # u8[kp, p, j, c] = fp8(U[kp*256 + j*128 + p, c] * S_U)
    u8 = np.clip(U * S_U, -240.0, 240.0).reshape(KT // 2, 2, 128, DIM)
    u8 = np.ascontiguousarray(u8.transpose(0, 2, 1, 3)).astype(fp8_np)
    # p_re[p, k*64:(k+1)*64] = P[k*128 + p, :]
    p_re = np.ascontiguousarray(
        P.astype(np.float32).reshape(KT, 128, K_EXP).transpose(1, 0, 2)
        .reshape(128, KT * K_EXP)
    )
    return u8, p_re


def build_program(ntok=NTOK, nrep=1):
    """One NeuronCore's program; identical across the 8 cores (SPMD).

    nrep > 1 wraps the whole computation in an on-device loop (used only by
    test.py to measure per-iteration HW time through the axon tunnel, whose
    per-execute overhead is ~100x the kernel itself).
    """
    D = DIM
    KT = D // 128          # contraction chunks
    TT = ntok // 128       # token tiles
    CT = D // 512          # 512-wide column tiles of U
    BLK = min(4, TT)       # token tiles per denominator block
    NBLK = TT // BLK
    NTB = max(1, ntok // 512)  # 512-token blocks for the transposed numerator

    nc = bacc.Bacc(None, target_bir_lowering=False)
    xt = nc.dram_tensor("xt", [D, ntok], F32, kind="ExternalInput")
    u8 = nc.dram_tensor("u8", [KT // 2, 128, 2, D], FP8, kind="ExternalInput")
    # P rearranged on host to [128, KT*64]: chunk k lives at [:, k*64:(k+1)*64]
    p = nc.dram_tensor("p", [128, KT * K_EXP], F32, kind="ExternalInput")
    scores_o = nc.dram_tensor("scores", [ntok, K_EXP], F32, kind="ExternalOutput")
    weights_o = nc.dram_tensor("weights", [ntok, 2], F32, kind="ExternalOutput")
    indices_o = nc.dram_tensor("indices", [ntok, 2], U32, kind="ExternalOutput")

    with TileContext(nc) as tc:
        with (
            tc.tile_pool(name="persist", bufs=1) as persist,
            tc.tile_pool(name="slab", bufs=3) as slab_pool,
            tc.tile_pool(name="ustream", bufs=8) as u_pool,
            tc.tile_pool(name="small", bufs=4) as small,
        ):
            # resident across the whole kernel: fp8(x.T * S_X) in the
            # DoubleRowSwInterleave weights layout -- for token tile t and
            # chunk pair kp, xsw[p, kp, t*256 + 254 - 2m + i] holds
            # x8[p, chunk 2kp+i, token t*128+m]
            xsw = persist.tile([128, KT // 2, TT * 256], FP8)
            p_sb = persist.tile([128, KT * K_EXP], F32)
            numT_sb = persist.tile([64, NTB * 512], F32)   # (x @ P).T
            num_sb = persist.tile([128, TT * K_EXP], F32)  # x @ P, token-major
            sq = persist.tile([128, 512], F32)
            ident = persist.tile([128, 128], F32)
            make_identity(nc, ident[:])

            loop_cm = tc.For_i(0, nrep, 1) if nrep > 1 else None
            if loop_cm is not None:
                loop_cm.__enter__()

            nc.sync.dma_start(p_sb[:], p[:])

            # ---- phase 0: stream x.T slabs, cast to fp8, fp32 numerator MMs
            # numerator computed transposed: numT[64, tok] += P_k.T @ xT_k
            with tc.tile_pool(name="psum_num", bufs=1, space="PSUM") as psum_num:
                num_ps = [
                    psum_num.tile([64, 512], F32, tag=f"nT{i}", name=f"nT{i}")
                    for i in range(NTB)
                ]
                for k in range(KT):
                    s32 = slab_pool.tile([128, ntok], F32, tag="s32")
                    nc.sync.dma_start(s32[:], xt[k * 128:(k + 1) * 128, :])
                    dst = (
                        xsw[:, k // 2, :]
                        .rearrange("p (t c) -> p t c", c=256)[:, :, 254 + (k % 2)::-2]
                    )
                    nc.vector.tensor_scalar_mul(
                        dst, s32.rearrange("p (t m) -> p t m", m=128), S_X
                    )
                    for b in range(NTB):
                        w = min(512, ntok - b * 512)
                        nc.tensor.matmul(
                            num_ps[b][:, :w],
                            p_sb[:, k * K_EXP:(k + 1) * K_EXP],
                            s32[:, b * 512:b * 512 + w],
                            start=(k == 0),
                            stop=(k == KT - 1),
                        )
                for b in range(NTB):
                    w = min(512, ntok - b * 512)
                    nc.vector.tensor_copy(
                        numT_sb[:, b * 512:b * 512 + w], num_ps[b][:, :w]
                    )
            # transpose numT [64, ntok] back to token-major num_sb [128, TT*64]
            with tc.tile_pool(name="psum_tp", bufs=2, space="PSUM") as psum_tp:
                for t in range(TT):
                    tp = psum_tp.tile([128, K_EXP], F32, tag="tp")
                    nc.tensor.transpose(
                        tp[:], numT_sb[:, t * 128:(t + 1) * 128], ident[:64, :64]
                    )
                    nc.vector.tensor_copy(
                        num_sb[:, t * K_EXP:(t + 1) * K_EXP], tp[:]
                    )

            # ---- phase 1: fp8 DoubleRow denominator matmul + epilogue
            psum_d_cm = tc.tile_pool(name="psum_d", bufs=8 // BLK, space="PSUM")
            psum_d = psum_d_cm.__enter__()
            for blk in range(NBLK):
                dparts = [
                    small.tile([128, CT], F32, tag=f"dp{t}", name=f"dp{t}")
                    for t in range(BLK)
                ]
                for col in range(CT):
                    pss = [
                        psum_d.tile([128, 512], F32, tag=f"ps{t}", name=f"ps{t}")
                        for t in range(BLK)
                    ]
                    for kp in range(KT // 2):
                        ut = u_pool.tile([128, 2, 512], FP8, tag="ut")
                        nc.sync.dma_start(
                            ut[:], u8[kp, :, :, col * 512:(col + 1) * 512]
                        )
                        for t in range(BLK):
                            tok = blk * BLK + t
                            nc.tensor.matmul(
                                pss[t][:],
                                xsw[:, kp, tok * 256:(tok + 1) * 256],
                                ut[:],
                                perf_mode=mybir.MatmulPerfMode.DoubleRowSwInterleave,
                                start=(kp == 0),
                                stop=(kp == KT // 2 - 1),
                            )
                    for t in range(BLK):
                        # sum of squares along free dim -> dparts[t][:, col]
                        nc.scalar.activation(
                            sq[:],
                            pss[t][:],
                            mybir.ActivationFunctionType.Square,
                            accum_out=dparts[t][:, col:col + 1],
                        )
                for t in range(BLK):
                    tok = blk * BLK + t
                    den = small.tile([128, 1], F32, tag="den")
                    nc.vector.tensor_reduce(
                        den[:], dparts[t][:], axis=mybir.AxisListType.X,
                        op=mybir.AluOpType.add,
                    )
                    # descale: den_true = sqrt(den_sum) / (S_X * S_U)
                    nc.scalar.activation(
                        den[:], den[:], mybir.ActivationFunctionType.Sqrt,
                        scale=1.0 / (S_X * S_U) ** 2,
                    )
                    rden = small.tile([128, 1], F32, tag="rden")
                    nc.vector.reciprocal(rden[:], den[:])
                    sc_t = small.tile([128, K_EXP], F32, tag="sc")
                    nc.vector.tensor_scalar_mul(
                        sc_t[:], num_sb[:, tok * K_EXP:(tok + 1) * K_EXP], rden[:]
                    )
                    nc.sync.dma_start(
                        scores_o[tok * 128:(tok + 1) * 128, :], sc_t[:]
                    )
                    m8 = small.tile([128, 8], F32, tag="m8")
                    i8 = small.tile([128, 8], U32, tag="i8")
                    nc.vector.max(m8[:], sc_t[:])
                    nc.vector.max_index(i8[:], m8[:], sc_t[:])
                    nc.sync.dma_start(
                        indices_o[tok * 128:(tok + 1) * 128, :], i8[:, 0:2]
                    )
                    dif = small.tile([128, 1], F32, tag="dif")
                    nc.vector.tensor_sub(dif[:], m8[:, 1:2], m8[:, 0:1])  # v1 - v0
                    wt = small.tile([128, 2], F32, tag="wt")
                    nc.scalar.activation(
                        wt[:, 0:1], dif[:], mybir.ActivationFunctionType.Sigmoid,
                        scale=-1.0,
                    )
                    nc.scalar.activation(
                        wt[:, 1:2], dif[:], mybir.ActivationFunctionType.Sigmoid,
                    )
                    nc.sync.dma_start(
                        weights_o[tok * 128:(tok + 1) * 128, :], wt[:]
                    )
            psum_d_cm.__exit__(None, None, None)
            if loop_cm is not None:
                loop_cm.__exit__(None, None, None)
    nc.compile()
    return nc


def _fp8_bias_correction(x, U, u8):
    """The fp8 quantization noise inflates den = ||x@U|| by a systematic
    ~sigma^2/2 (the random part averages out over 4096 columns).  Estimate
    the ratio on a token sample and return c = mean(den_exact / den_fp8),
    which the caller folds into P (scores = num/den, so scaling num by c
    corrects scores without touching the device program)."""
    rows = np.linspace(0, x.shape[0] - 1, 64).astype(int)
    xs = x[rows].astype(np.float64)
    fp8_np = u8.dtype
    xs8 = (xs * S_X).astype(fp8_np).astype(np.float64)
    # u8 is [KT//2, 128, 2, D] with U row kp*256 + j*128 + p at [kp, p, j, :]
    KT = DIM // 128
    u8f = u8.astype(np.float64).transpose(0, 2, 1, 3).reshape(DIM, DIM)
    den_fp8 = np.linalg.norm(xs8 @ u8f, axis=-1) / (S_X * S_U)
    den_exact = np.linalg.norm(xs @ U, axis=-1)
    return float(np.mean(den_exact / den_fp8))


def kernel(x, proj_w, positions, theta, signatures):
    global LAST_RESULTS
    x = np.asarray(x)
    proj_w = np.asarray(proj_w)
    positions = np.asarray(positions)
    theta = np.asarray(theta)
    signatures = np.asarray(signatures)
    U, P = _host_precompute(proj_w, positions, theta, signatures)
    u8, p_re = host_arrays(U, P)
    p_re = p_re * np.float32(_fp8_bias_correction(x, U, u8))
    xt_full = np.ascontiguousarray(x.astype(np.float32).T)  # [HIDDEN, N_TOKENS]

    key = (NTOK, 1)
    if key not in _PROGRAM_CACHE:
        _PROGRAM_CACHE[key] = build_program(NTOK)
    nc = _PROGRAM_CACHE[key]

    in_maps = []
    for c in range(N_CORES):
        sl = np.ascontiguousarray(xt_full[:, c * NTOK:(c + 1) * NTOK])
        in_maps.append({"xt": sl, "u8": u8, "p": p_re})

    LAST_RESULTS = run_bass_kernel_spmd(nc, in_maps, list(range(N_CORES)))
    results = LAST_RESULTS.results

    weights = np.concatenate([r["weights"] for r in results], axis=0)
    indices = np.concatenate(
        [r["indices"].astype(np.int32) for r in results], axis=0
    )
    scores = np.concatenate([r["scores"] for r in results], axis=0)
    return weights.astype(np.float32), indices, scores.astype(np.float32)
